# revision 13
# baseline (speedup 1.0000x reference)
"""Trainium2 Bass kernel for nn_EquiConv2d (equirectangular deformable conv).

Key structural facts exploited (derived from the reference geometry):
  * off_y is exactly longitude-invariant, so each (tap k, row h) samples two
    fixed input rows (iy0, iy0+1) with a constant y-fraction.
  * off_x is longitude-invariant up to the 2*pi wrap: sampling along a row is
    a CIRCULAR shift by a constant s0(k,h) plus a constant x-fraction.
  * Hence the whole deformable conv is a set of matmuls per output row
    ([128=(c x row-pair) contraction, 512 free]) reading circularly
    duplicated row-pair tiles at per-(k,h) column offsets, with the bilinear
    corner weights folded into the stationary (weight) operand.
  * ROW-PAIR MERGING: adjacent output rows (2p, 2p+1) share most (event,
    region, offset) reads (a 3x3 kernel makes each input row serve ~3 output
    rows).  Both rows' outputs live in ONE [128, 512] PSUM bank (row 2p in
    partitions 0-63, row 2p+1 in 64-127); a shared slot is a single matmul
    with lhsT [128, 128] (each row's coefficient-scaled weights in its own
    column half).  Matmul cost depends only on the streamed free size, so
    shared slots compute both rows for the price of one: ~670 matmuls per
    core instead of ~955.
  * Two fp32 oddities handled exactly: tap (k=7,h=255) is identically zero
    and tap (k=1,h=1) samples near the antipode with fp32-noise-scattered
    positions -> 3 extra matmul slots with per-column coefficient vectors
    (active only on the cores owning global row 1).
  * Output is written f16 (halves store traffic; ~4e-4 relative rounding,
    well within tolerance) and widened to fp32 on the host.

Sharding: 8 cores = 2 batches x 4 bands of 64 output rows (32 row-pairs).
"""

import math

import numpy as np

# ----------------------------------------------------------------------------
# problem constants
B, C, H, W = 2, 64, 256, 512
O, KH, KW = 64, 3, 3
K = KH * KW
NCORES = 8
NROW = 64            # output rows per core
NPAIR = NROW // 2    # merged row-pairs per core
NSPEC = 3            # special (antipode) slots, accumulated into local row 1
RING = 16            # staged row-pair ring slots
PFP = 2              # staging prefetch lead (pairs)
PRESTAGE = 5         # events staged uniformly before the band dispatch
SLOTW = 2048         # F(1024) + G(1024) columns per ring slot
GOFF = 1024
SKIP_TOL = 1e-4      # drop matmul slots with |coeff| below this
LTMAX = 2048         # per-pair lhsT tile columns (max actual is 1920)

_CACHE = {}


# ----------------------------------------------------------------------------
# host-side geometry tables (must replicate reference fp32 semantics exactly)

def _compute_offsets_jax():
    """Bit-exact replica of reference.equi_offsets on jax CPU."""
    import jax
    import jax.numpy as jnp
    cpu = jax.devices("cpu")[0]
    with jax.default_device(cpu):
        dtype = jnp.float32
        pano_H, pano_W, kH, kW = H, W, KH, KW
        Kk = kH * kW
        u = jnp.arange(pano_W, dtype=dtype)
        v = jnp.arange(pano_H, dtype=dtype)
        phi = (u - pano_W / 2.0) / pano_W * (2.0 * math.pi)
        theta = -(v - pano_H / 2.0) / pano_H * math.pi
        cp, sp = jnp.cos(phi), jnp.sin(phi)
        z, one = jnp.zeros_like(cp), jnp.ones_like(cp)
        Ry = jnp.stack([jnp.stack([cp, z, sp], -1),
                        jnp.stack([z, one, z], -1),
                        jnp.stack([-sp, z, cp], -1)], -2)
        ct, st = jnp.cos(theta), jnp.sin(theta)
        zh, oh = jnp.zeros_like(ct), jnp.ones_like(ct)
        Rx = jnp.stack([jnp.stack([oh, zh, zh], -1),
                        jnp.stack([zh, ct, -st], -1),
                        jnp.stack([zh, st, ct], -1)], -2)
        ROT = jnp.einsum('wij,hjk->hwik', Ry, Rx)
        fov_w = kW * (2.0 * math.pi / pano_W)
        focal = (kW / 2.0) / math.tan(fov_w / 2.0)
        hg = (jnp.arange(kH, dtype=dtype)[:, None] + 0.5 - kH / 2.0)
        wg = (jnp.arange(kW, dtype=dtype)[None, :] + 0.5 - kW / 2.0)
        hg = jnp.broadcast_to(hg, (kH, kW)).reshape(Kk)
        wg = jnp.broadcast_to(wg, (kH, kW)).reshape(Kk)
        rays0 = jnp.stack([wg / focal, hg / focal, jnp.ones(Kk, dtype)], 0)
        rays0 = rays0 / jnp.linalg.norm(rays0, axis=0, keepdims=True)
        rays = jnp.einsum('hwik,kn->hwin', ROT, rays0)
        phi2 = jnp.arctan2(rays[..., 0, :], rays[..., 2, :])
        th2 = jnp.arcsin(jnp.clip(rays[..., 1, :], -1.0, 1.0))
        x = pano_W / (2.0 * math.pi) * phi2 + pano_W / 2.0
        y = pano_H / math.pi * th2 + pano_H / 2.0
        off_x = x - (wg[None, None, :] + u[None, :, None])
        off_y = y - (hg[None, None, :] + v[:, None, None])
        return (np.asarray(jnp.transpose(off_y, (2, 0, 1))),
                np.asarray(jnp.transpose(off_x, (2, 0, 1))))


def _build_tap_tables():
    off_y, off_x = _compute_offsets_jax()
    ky = np.repeat(np.arange(KH), KW).astype(np.float32)
    kx = np.tile(np.arange(KW), KH).astype(np.float32)
    base_x = (np.arange(W, dtype=np.float32) - np.float32(1))
    base_y = (np.arange(H, dtype=np.float32) - np.float32(1))
    px = (base_x[None, None, :] + kx[:, None, None] + off_x).astype(np.float32)
    py = (base_y[None, :, None] + ky[:, None, None] + off_y).astype(np.float32)
    pyc = py[:, :, 0]
    assert np.all(py == pyc[:, :, None]), "off_y not longitude-invariant"

    iy0 = np.floor(pyc).astype(np.int64)
    wy1 = (pyc - np.floor(pyc)).astype(np.float64)
    v0 = (iy0 >= 0) & (iy0 < H)
    v1 = (iy0 + 1 >= 0) & (iy0 + 1 < H)
    cy0 = np.where(v0, 1.0 - wy1, 0.0)
    cy1 = np.where(v1, wy1, 0.0)

    Draw = np.mod((px.astype(np.float64) - np.arange(W)[None, None, :]), 512.0)
    ang = Draw / 512.0 * 2 * np.pi
    mean = np.mod(np.angle(np.exp(1j * ang).mean(axis=2)) / (2 * np.pi) * 512.0,
                  512.0)
    resid = np.mod(Draw - mean[:, :, None] + 256.0, 512.0) - 256.0
    D = mean + np.median(resid, axis=2)
    s0 = np.mod(np.floor(D), 512).astype(np.int64)
    frac = D - np.floor(D)

    special = np.zeros((K, H), dtype=bool)
    special[1, 1] = True
    dead = (cy0 == 0.0) & (cy1 == 0.0)

    Ddev = np.abs(np.mod(Draw - D[:, :, None] + 256.0, 512.0) - 256.0)
    dev = Ddev.max(axis=2)
    bad = (dev > 5e-4) & ~special & ~dead
    assert not bad.any(), f"unrepresentable taps: {np.argwhere(bad)}"

    def ref_coefs(p):
        x0 = math.floor(p)
        fr = p - x0
        out = {}
        for ix, wt in ((x0, 1.0 - fr), (x0 + 1, fr)):
            if 0 <= ix < W and wt != 0.0:
                out[ix] = out.get(ix, 0.0) + wt
        return out

    # seam variant selection: decided by the exact fp32 px at the wrap column
    slot0_useG = np.zeros((K, H), dtype=bool)
    slot1_useF = np.zeros((K, H), dtype=bool)
    for k in range(K):
        for h in range(H):
            if special[k, h] or dead[k, h]:
                continue
            s = int(s0[k, h]); fr = frac[k, h]
            if s >= 1:
                w0 = (512 - s) % 512
                rc = ref_coefs(float(px[k, h, w0]))
                slot0_useG[k, h] = (abs(rc.get(0, 0.0))
                                    < abs(rc.get(0, 0.0) - (1 - fr)))
            w1 = (511 - s) % 512
            rc = ref_coefs(float(px[k, h, w1]))
            slot1_useF[k, h] = (abs(rc.get(0, 0.0) - fr)
                                < abs(rc.get(0, 0.0)))

    # special tap (1,1): per-column coefficients on F offsets 255..257
    pxs = px[1, 1, :].astype(np.float64)
    Gam = np.zeros((3, W), dtype=np.float64)
    for w in range(W):
        p = pxs[w]
        x0 = math.floor(p)
        fr = p - x0
        for ix, wt in ((x0, 1.0 - fr), (x0 + 1, fr)):
            if 0 <= ix < W and wt != 0.0:
                found = False
                for jj in range(3):
                    if (255 + jj + w) % 512 == ix % 512:
                        Gam[jj, w] += wt
                        found = True
                        break
                assert found, (w, p, ix)

    return dict(iy0=iy0, cy0=cy0, cy1=cy1, s0=s0, frac=frac,
                slot0_useG=slot0_useG, slot1_useF=slot1_useF,
                special=special, dead=dead, Gam=Gam)


# ----------------------------------------------------------------------------
# per-row slot lists and the pair-merged SPMD schedule

def _slots_of(tt, h):
    """Map (event_row, region, offset) -> list of (k, coefLo, coefHi).

    coefLo/coefHi scale the weight copies for input rows iy0 / iy0+1
    (partitions 0-63 / 64-127 of the staged row-pair tile)."""
    out = {}
    for k in range(K):
        if tt['dead'][k, h] or tt['special'][k, h]:
            continue
        fr = tt['frac'][k, h]
        s = int(tt['s0'][k, h])
        r = int(np.clip(tt['iy0'][k, h], 0, 255))
        c0, c1 = tt['cy0'][k, h], tt['cy1'][k, h]
        if (1.0 - fr) * max(c0, c1) >= SKIP_TOL:
            key = (r, 'G', s - 1) if (tt['slot0_useG'][k, h] and s >= 1) \
                else (r, 'F', s)
            out.setdefault(key, []).append((k, c0 * (1 - fr), c1 * (1 - fr)))
        if fr * max(c0, c1) >= SKIP_TOL:
            key = (r, 'F', s + 1) if tt['slot1_useF'][k, h] \
                else (r, 'G', s)
            out.setdefault(key, []).append((k, c0 * fr, c1 * fr))
    return out


def _build_schedule(tt):
    """Per band: staged events (row-pairs) + per-pair merged matmul lists."""
    blocks = []
    for blk in range(4):
        h0 = blk * NROW
        ev_of, events, first_use = {}, [], []
        pairs = []
        for p in range(NPAIR):
            A = _slots_of(tt, h0 + 2 * p)
            Bm = _slots_of(tt, h0 + 2 * p + 1)
            for key in list(A.keys()) + [k2 for k2 in Bm if k2 not in A]:
                r = key[0]
                if r not in ev_of:
                    ev_of[r] = len(events)
                    events.append(r)
                    first_use.append(p)
            shared = sorted(set(A) & set(Bm))
            lo = sorted(set(A) - set(Bm))
            hi = sorted(set(Bm) - set(A))
            assert len(shared) >= 2, (blk, p, len(shared))
            # emit order: shared[0] (start) ... singles ... shared[-1] (stop)
            emits = []
            for key in [shared[0]] + shared[1:-1]:
                emits.append(('SH', key, A[key], Bm[key]))
            for key in lo:
                emits.append(('LO', key, A[key], None))
            for key in hi:
                emits.append(('HI', key, None, Bm[key]))
            emits.append(('SH', shared[-1], A[shared[-1]], Bm[shared[-1]]))
            pairs.append(dict(emits=emits, ev_of=dict(ev_of)))
        need = []
        for p in range(NPAIR):
            need.append([ev_of[key[0]] for _t, key, _a, _b in
                         pairs[p]['emits']])
        ev_spec = -1
        if blk == 0:
            r_spec = int(np.clip(tt['iy0'][1, 1], 0, 255))
            assert r_spec in ev_of, "special event row not staged"
            ev_spec = ev_of[r_spec]
            assert first_use[ev_spec] == 0
            need[0].append(ev_spec)
        blocks.append(dict(events=events, first_use=first_use,
                           pairs=pairs, need=need, ev_of=ev_of,
                           ev_spec=ev_spec))

    E = max(len(b['events']) for b in blocks)
    for b in blocks:
        while len(b['events']) < E:
            b['events'].append(b['events'][-1])

    # staged-count target before pair p (uniform enough per band; emitted
    # per-band statically anyway)
    for b in blocks:
        fu = np.asarray(b['first_use'])
        U = [int(np.searchsorted(fu, p, 'right')) for p in range(NPAIR)]
        tgt = [U[min(p + PFP, NPAIR - 1)] for p in range(NPAIR)]
        tgt[0] = U[0]
        for p in range(1, NPAIR):
            tgt[p] = max(tgt[p], tgt[p - 1])
        b['tgt'] = tgt
        # ring-overwrite feasibility: staging event e (ring slot e%RING)
        # happens just before the first pair with tgt > e; event e-RING
        # must have been fully consumed by then.
        E_b = len(b['events'])
        ls = [NPAIR] * E_b
        for e in range(E_b):
            for p in range(NPAIR):
                if b['tgt'][p] > e:
                    ls[e] = p
                    break
        lastuse = {}
        for p in range(NPAIR):
            for e in b['need'][p]:
                lastuse[e] = p
        for e in range(RING, E_b):
            if e - RING in lastuse:
                assert lastuse[e - RING] < ls[e], \
                    f"RING={RING} too small: ev{e} overwrites ev{e-RING}"
    return blocks, E


def _key_v(key, eidx):
    r, reg, off = key
    base = (eidx % RING) * SLOTW
    return base + off + (GOFF if reg == 'G' else 0)


def _build_lt(tt, blocks, weight):
    """Per-band flat lhsT tables + per-pair emit metadata.

    Returns (lt_arrays[4] as [128, LTTOT] f16, meta[4]) where meta[blk][p] is
    (col0, ncols, list of (colOff, width, partOff, v, start, stop))."""
    w3 = weight.reshape(O, C, K).astype(np.float64)
    wko = w3.transpose(1, 2, 0)                      # [C, K, O]

    lts, metas, tots = [], [], []
    for blk in range(4):
        b = blocks[blk]
        cols = []
        meta = []
        col0 = 0
        for p in range(NPAIR):
            emits = b['pairs'][p]['emits']
            ne = len(emits)
            entries = []
            c = 0
            for i, (typ, key, la, lb) in enumerate(emits):
                eidx = b['ev_of'][key[0]]
                v = _key_v(key, eidx)
                start = (i == 0)
                stop = (i == ne - 1)
                if typ == 'SH':
                    blkcols = np.zeros((128, 128), np.float64)
                    for kk, cc0, cc1 in la:
                        blkcols[:C, :O] += cc0 * wko[:, kk, :]
                        blkcols[C:, :O] += cc1 * wko[:, kk, :]
                    for kk, cc0, cc1 in lb:
                        blkcols[:C, O:] += cc0 * wko[:, kk, :]
                        blkcols[C:, O:] += cc1 * wko[:, kk, :]
                    width, po = 128, 0
                else:
                    blkcols = np.zeros((128, 64), np.float64)
                    lab = la if typ == 'LO' else lb
                    for kk, cc0, cc1 in lab:
                        blkcols[:C, :] += cc0 * wko[:, kk, :]
                        blkcols[C:, :] += cc1 * wko[:, kk, :]
                    width, po = 64, (0 if typ == 'LO' else 64)
                cols.append(blkcols)
                entries.append((c, width, po, v, start, stop))
                c += width
            assert c <= LTMAX, (blk, p, c)
            meta.append((col0, c, entries))
            col0 += c
        lt = np.concatenate(cols, axis=1).astype(np.float16)
        lts.append(lt)
        metas.append(meta)
        tots.append(col0)
    LTTOT = max(tots)
    lts = [np.ascontiguousarray(
             np.pad(lt, ((0, 0), (0, LTTOT - lt.shape[1]))))
           for lt in lts]
    return lts, metas, LTTOT


# ----------------------------------------------------------------------------
# device program

def _emit_section(tc, aps, tiles, blkinfo, meta, j):
    """Emit one per-band section (all-static APs)."""
    import concourse.mybir as mybir
    nc = tc.nc
    f16 = mybir.dt.float16
    f32 = mybir.dt.float32
    buf, coeft, biast, ltst = tiles
    xb, outd, lt = aps['xb'], aps['out'], aps['lt']

    bufR = buf.rearrange("p (r s) -> p r s", r=RING)

    def stage_chunk(e0, e1):
        """Stage events [e0, e1) (no ring wrap inside): one DMA (F half,
        duplicated via Pool copy) + one wide Vector copy for the shifted G
        half + the G seam-zero memset."""
        n = e1 - e0
        r0 = e0 % RING
        src = xb[e0:e1].rearrange("e p c w -> (p c) e w")
        nc.sync.dma_start(bufR[:, r0:r0 + n, 0:W], src)
        nc.vector.tensor_copy(bufR[:, r0:r0 + n, W:2 * W],
                              bufR[:, r0:r0 + n, 0:W])
        nc.vector.tensor_copy(bufR[:, r0:r0 + n, GOFF:GOFF + 1023],
                              bufR[:, r0:r0 + n, 1:1024])
        nc.gpsimd.memset(bufR[:, r0:r0 + n, GOFF + 511:GOFF + 512], 0.0)

    psp, ltp, zp, outp = tiles_pools[0]

    staged = PRESTAGE
    for p in range(NPAIR):
        tgt = blkinfo['tgt'][p]
        while staged < tgt:
            e1 = min(tgt, staged + 8,
                     (staged // RING + 1) * RING)
            stage_chunk(staged, e1)
            staged = e1
        col0, ncols, entries = meta[p]
        if p == 0:
            ltt = aps['ltt0']
        else:
            ltt = ltp.tile([128, LTMAX], f16, tag="ltt")
            nc.scalar.dma_start(ltt[:, :ncols], lt[:, col0:col0 + ncols])
        ps = psp.tile([128, W], f32, tag="ps")
        nsp = NSPEC if (j == 0 and p == 0) else 0
        ne = len(entries)
        for i, (c, width, po, v, start, stop) in enumerate(entries):
            # specials are injected just before the closing shared slot
            if nsp and i == ne - 1:
                sbase = (blkinfo['ev_spec'] % RING) * SLOTW
                for jj in range(NSPEC):
                    zt = zp.tile([128, W], f16, tag="spz")
                    nc.vector.tensor_mul(
                        zt, buf[:, sbase + 255 + jj:sbase + 255 + jj + W],
                        coeft[:, jj * W:(jj + 1) * W])
                    nc.tensor.matmul(ps[64:128, :],
                                     ltst[:, jj * O:(jj + 1) * O], zt,
                                     start=False, stop=False)
            nc.tensor.matmul(ps[po:po + width, :], ltt[:, c:c + width],
                             buf[:, v:v + W], start=start, stop=stop)
        ot = outp.tile([128, W], f16, tag="out")
        nc.scalar.activation(ot, ps,
                             mybir.ActivationFunctionType.Identity,
                             bias=biast, scale=1.0)
        nc.scalar.dma_start(outd[p], ot)


tiles_pools = [None]


def _emit_kernel(tc, aps, blocks, metas):
    import concourse.mybir as mybir
    nc = tc.nc
    f16 = mybir.dt.float16
    f32 = mybir.dt.float32

    with tc.tile_pool(name="bigp", bufs=1) as bigp, \
         tc.tile_pool(name="ltp", bufs=3) as ltp, \
         tc.tile_pool(name="zp", bufs=3) as zp, \
         tc.tile_pool(name="psp", bufs=4, space="PSUM") as psp, \
         tc.tile_pool(name="outp", bufs=3) as outp:

        buf = bigp.tile([128, RING * SLOTW], f16)
        coeft = bigp.tile([128, NSPEC * W], f16)
        biast = bigp.tile([128, 1], f32)
        ltst = bigp.tile([128, NSPEC * O], f16)

        # uniform (band-independent) prestage of the first events and the
        # first pair's lhsT table: issued before the dispatch so the DMA
        # ring-init and transfers overlap the per-engine value loads.
        bufR = buf.rearrange("p (r s) -> p r s", r=RING)
        src = aps['xb'][0:PRESTAGE].rearrange("e p c w -> (p c) e w")
        nc.sync.dma_start(bufR[:, 0:PRESTAGE, 0:W], src)
        nc.vector.tensor_copy(bufR[:, 0:PRESTAGE, W:2 * W],
                              bufR[:, 0:PRESTAGE, 0:W])
        nc.vector.tensor_copy(bufR[:, 0:PRESTAGE, GOFF:GOFF + 1023],
                              bufR[:, 0:PRESTAGE, 1:1024])
        nc.gpsimd.memset(bufR[:, 0:PRESTAGE, GOFF + 511:GOFF + 512], 0.0)

        ltt0 = ltp.tile([128, LTMAX], f16, tag="ltt")
        nc.scalar.dma_start(ltt0, aps['lt'][:, 0:LTMAX])
        nc.scalar.dma_start(biast, aps['biasd'])
        nc.scalar.dma_start(coeft, aps['coefr'])
        nc.scalar.dma_start(ltst, aps['lts'])
        aps = dict(aps, ltt0=ltt0)

        blkv = nc.values_load(aps['blkid'][0:1, 0:1],
                              min_val=0, max_val=3,
                              skip_runtime_bounds_check=True)

        tiles = (buf, coeft, biast, ltst)
        tiles_pools[0] = (psp, ltp, zp, outp)
        for j in tc.Switch(blkv, 4):
            _emit_section(tc, aps, tiles, blocks[j], metas[j], j)


def _get_compiled():
    """Build tables, schedule, and the Bass program once."""
    if 'prog' in _CACHE:
        return _CACHE['prog']
    import concourse.mybir as mybir
    import concourse.tile as tile
    from concourse import bacc

    tt = _build_tap_tables()
    blocks, E = _build_schedule(tt)
    # lt layout (column ranges) depends only on geometry, not weights; build
    # once with dummy weights to size the dram tensor.
    _lts, metas, LTTOT = _build_lt(tt, blocks, np.zeros((O, C, KH, KW),
                                                        np.float32))

    f16 = mybir.dt.float16
    f32 = mybir.dt.float32
    nc = bacc.Bacc("TRN2", target_bir_lowering=False, debug=False,
                   num_devices=NCORES)
    aps = {
        'xb': nc.dram_tensor("xb", [E, 2, C, W], f16,
                             kind="ExternalInput").ap(),
        'lt': nc.dram_tensor("lt", [128, LTTOT], f16,
                             kind="ExternalInput").ap(),
        'lts': nc.dram_tensor("lts", [128, NSPEC * O], f16,
                              kind="ExternalInput").ap(),
        'blkid': nc.dram_tensor("blkid", [1, 1], mybir.dt.int32,
                                kind="ExternalInput").ap(),
        'coefr': nc.dram_tensor("coefr", [128, NSPEC * W], f16,
                                kind="ExternalInput").ap(),
        'biasd': nc.dram_tensor("biasd", [128, 1], f32,
                                kind="ExternalInput").ap(),
        'out': nc.dram_tensor("out", [NPAIR, 128, W], f16,
                              kind="ExternalOutput").ap(),
    }
    with tile.TileContext(nc) as tc:
        _emit_kernel(tc, aps, blocks, metas)
    nc.finalize()

    _CACHE['prog'] = (nc, tt, blocks, E, metas, LTTOT)
    return _CACHE['prog']


def _core_inputs(x, weight, bias, tt, blocks, E):
    """Assemble per-core in_maps. Core c = batch (c // 4), band (c % 4)."""
    lts_arr, _metas, _LTTOT = _build_lt(tt, blocks, weight)

    w3 = weight.reshape(O, C, K).astype(np.float64)
    wkodup = np.empty((128, K, O), np.float64)
    wkodup[:C] = w3.transpose(1, 2, 0)
    wkodup[C:] = w3.transpose(1, 2, 0)

    biasd = np.ascontiguousarray(
        np.concatenate([bias, bias]).reshape(128, 1).astype(np.float32))

    lts_on = np.zeros((128, NSPEC * O), np.float16)
    for jj in range(NSPEC):
        lts_on[:C, jj * O:(jj + 1) * O] = wkodup[:C, 1, :].astype(np.float16)
    lts_off = np.zeros((128, NSPEC * O), np.float16)

    Gam = tt['Gam'].astype(np.float16)
    coef_on = np.ascontiguousarray(
        np.broadcast_to(Gam[:, None, :], (NSPEC, 128, W))
        .transpose(1, 0, 2).reshape(128, NSPEC * W))
    coef_off = np.zeros((128, NSPEC * W), np.float16)

    in_maps = []
    for cid in range(NCORES):
        b, blk = cid // 4, cid % 4
        xz = np.concatenate([x[b], np.zeros((C, 1, W), x.dtype)], axis=1)
        xz = xz.astype(np.float16)
        rows = np.asarray(blocks[blk]['events'], np.int64)
        pair_idx = np.stack([rows, rows + 1], axis=1)       # [E, 2]
        xbv = xz[:, pair_idx, :]                            # [C, E, 2, W]
        xbv = np.ascontiguousarray(xbv.transpose(1, 2, 0, 3))  # [E,2,C,W]
        in_maps.append({
            'xb': xbv,
            'lt': lts_arr[blk],
            'lts': lts_on if blk == 0 else lts_off,
            'blkid': np.array([[blk]], np.int32),
            'coefr': coef_on if blk == 0 else coef_off,
            'biasd': biasd,
        })
    return in_maps


def _gather_output(res):
    out = np.empty((B, O, H, W), np.float32)
    for cid in range(NCORES):
        b, blk = cid // 4, cid % 4
        oc = np.asarray(res.results[cid]['out'], np.float32)  # [NPAIR,128,W]
        h0 = blk * NROW
        out[b, :, h0 + 0:h0 + NROW:2, :] = oc[:, :O, :].transpose(1, 0, 2)
        out[b, :, h0 + 1:h0 + NROW:2, :] = oc[:, O:, :].transpose(1, 0, 2)
    return out


def kernel(x, weight, bias):
    from concourse.bass_utils import run_bass_kernel_spmd
    x = np.asarray(x, dtype=np.float32)
    weight = np.asarray(weight, dtype=np.float32)
    bias = np.asarray(bias, dtype=np.float32)

    nc, tt, blocks, E, _metas, _LTTOT = _get_compiled()
    in_maps = _core_inputs(x, weight, bias, tt, blocks, E)
    res = run_bass_kernel_spmd(nc, in_maps, core_ids=list(range(NCORES)))
    return _gather_output(res)
